# revision 15
# baseline (speedup 1.0000x reference)
"""Trainium2 Bass kernel for nn_BasicBlock (binarized ResNet basic block).

Computation (see problem reference):
    residual = x
    out = psum_conv3x3(sign(x), sign(w1))        # 3x3 'same' conv, saturating acc
    out = bn1(out); out = hardtanh(out)
    out = psum_conv3x3(sign(out), sign(w2))
    out = bn2(out); out = out + residual; out = hardtanh(out)

Key facts exploited:
  * C=128 channels = one GROUP, 9 taps of |partial| <= 128 each, so the
    running accumulator magnitude is <= 9*128 = 1152 < THRESH=8000: the
    saturation clip NEVER binds. The conv is a plain 3x3 conv over sign
    values, all arithmetic exact small integers -> freely reorderable and
    exactly representable in fp8e4/bf16 inputs with fp32 PSUM accumulation.
  * sign(hardtanh(v)) == sign(v), so the first hardtanh can be folded into
    the sign feeding conv2.
  * Each conv = 9 shifted-window taps (K=C=128 on partitions) into one PSUM
    accumulation group over a zero-padded row-stride-64 fp8 sign image:
    the 6 taps of rows 0..1 run as 3 fp8 DoubleRow matmuls (two vertically
    adjacent taps share one matmul: planes at +64 elements), the 3 taps of
    row 2 as normal fp8 matmuls.

Sharding: data-parallel over batch: 64 images -> 8 cores x 8 images.
"""

import sys

sys.path.insert(0, "/opt/trn_rl_repo")

import numpy as np
import ml_dtypes

import concourse.bass as bass
import concourse.bacc as bacc
import concourse.mybir as mybir
import concourse.tile as tile
from concourse.bass_utils import run_bass_kernel_spmd

# ---------------------------------------------------------------- constants

N_CORES = 8
B, C, H, W = 64, 128, 56, 56
BL = B // N_CORES            # images per core
HP = H + 2                   # padded rows
RW = 64                      # padded row width (stride): 56 valid + pads,
                             # 64 so the DoubleRow plane step (+RW) is 16-aligned
CHUNK_ROWS = 8               # output rows per PSUM chunk
NFLAT = CHUNK_ROWS * RW      # 512 flat psum columns per chunk (one bank)
N_CHUNKS = H // CHUNK_ROWS   # 7
EPS = 1e-5
SHIFT = (H + 2) * RW         # offset of the col-shifted copy inside xs/ts
WCOLS = 2 * (4 * 256 + 128)  # fp8 weight table columns (2 convs x 1152)

F32 = mybir.dt.float32
FP8 = mybir.dt.float8e4

_NC_CACHE = None


def _build_nc():
    """Build the per-core Bass module (same NEFF on all 8 cores)."""
    nc = bacc.Bacc("TRN2", debug=False)

    x_d = nc.dram_tensor("x", [BL, C, H, W], F32, kind="ExternalInput").ap()
    # host-prepped fp8 weight tables, per conv: 3 DoubleRow pair tables
    # [cin, 2*cout] for (r0,r1) at c=0,1,2 then 3 normal tables [cin, cout]
    # for r=2 at c=0,1,2
    w_d = nc.dram_tensor("w", [C, WCOLS], FP8, kind="ExternalInput").ap()
    # folded BN params per channel: [:,0]=inv1 [:,1]=b1 [:,2]=inv2 [:,3]=b2
    bn_d = nc.dram_tensor("bn", [C, 4], F32, kind="ExternalInput").ap()
    y_d = nc.dram_tensor("y", [BL, C, H, W], F32, kind="ExternalOutput").ap()

    SIGN = mybir.ActivationFunctionType.Sign
    DR = mybir.MatmulPerfMode.DoubleRow

    with tile.TileContext(nc) as tc:
        with (
            tc.tile_pool(name="const", bufs=1) as cpool,
            tc.tile_pool(name="xf32", bufs=2) as xpool,
            tc.tile_pool(name="xsign", bufs=2) as spool,
            tc.tile_pool(name="tmid", bufs=2) as tpool,
            tc.tile_pool(name="outs", bufs=3) as opool,
            tc.tile_pool(name="psum", bufs=3, space="PSUM") as pspool,
        ):
            w_sb = cpool.tile([C, WCOLS], FP8)
            nc.sync.dma_start(w_sb[:], w_d[:])
            bn_sb = cpool.tile([C, 4], F32)
            nc.sync.dma_start(bn_sb[:], bn_d[:])

            def conv_chunk(ps, src, conv_idx, h0):
                """One output chunk: 4 DoubleRow + 1 normal fp8 matmuls.

                DR c=0..2 pairs the vertically adjacent taps (r0,c)+(r1,c)
                (planes at +RW). DR #3 pairs (r2,c0)+(r2,c1) using the
                col-shifted copy at +SHIFT. Tap (r2,c2) is a normal matmul.
                """
                co = conv_idx * 1152
                ps3 = ps.rearrange("p (h w) -> p h w", w=RW)
                pout = ps3[:, :, 0:W]
                for c in range(3):
                    rhs = bass.AP(
                        tensor=src.tensor,
                        offset=src.offset + h0 * RW + c,
                        ap=[src.ap[0], [RW, 2], [RW, CHUNK_ROWS], [1, W]],
                    )
                    lhsT = w_sb[:, co + c * 256 : co + (c + 1) * 256].rearrange(
                        "p (j m) -> p j m", j=2
                    )
                    nc.tensor.matmul(
                        pout, lhsT, rhs, start=(c == 0), stop=False,
                        perf_mode=DR, skip_group_check=True,
                    )
                rhs = bass.AP(
                    tensor=src.tensor,
                    offset=src.offset + (h0 + 2) * RW,
                    ap=[src.ap[0], [SHIFT, 2], [RW, CHUNK_ROWS], [1, W]],
                )
                lhsT = w_sb[:, co + 768 : co + 1024].rearrange(
                    "p (j m) -> p j m", j=2
                )
                nc.tensor.matmul(
                    pout, lhsT, rhs, start=False, stop=False,
                    perf_mode=DR, skip_group_check=True,
                )
                rhs = bass.AP(
                    tensor=src.tensor,
                    offset=src.offset + (h0 + 2) * RW + 2,
                    ap=[src.ap[0], [RW, CHUNK_ROWS], [1, W]],
                )
                nc.tensor.matmul(
                    pout, w_sb[:, co + 1024 : co + 1152],
                    rhs, start=False, stop=True, skip_group_check=True,
                )

            def shift_copy(buf, row0, nrows):
                """Build sh[h, w] = main[h, w+1] for rows [row0, row0+nrows)."""
                src = bass.AP(
                    tensor=buf.tensor,
                    offset=buf.offset + row0 * RW + 1,
                    ap=[buf.ap[0], [1, nrows * RW]],
                )
                dst = bass.AP(
                    tensor=buf.tensor,
                    offset=buf.offset + SHIFT + row0 * RW,
                    ap=[buf.ap[0], [1, nrows * RW]],
                )
                nc.gpsimd.tensor_copy(dst, src)

            def zero_pads(buf, buf3):
                nc.gpsimd.memset(buf3[:, 0, :], 0.0)
                nc.gpsimd.memset(buf3[:, HP - 1, :], 0.0)
                nc.gpsimd.memset(buf3[:, 1 : HP - 1, 0:1], 0.0)
                nc.gpsimd.memset(buf3[:, 1 : HP - 1, W + 1 : RW], 0.0)
                # last padded row of the shifted copy is never built by
                # shift_copy; it is all pad-derived, so zero it once
                nc.gpsimd.memset(buf[:, SHIFT + (HP - 1) * RW : 2 * SHIFT], 0.0)

            for i in range(BL):
                # DMA + sign in 14-row pieces so conv1 can start after the
                # first piece instead of after the whole 1.6 MB transfer.
                x_sb = xpool.tile([C, H, W], F32)
                xs = spool.tile([C, 2 * HP * RW], FP8)
                xs3 = xs[:, 0:SHIFT].rearrange("p (h w) -> p h w", w=RW)
                zero_pads(xs, xs3)
                r0 = 0
                for nr in (10, 16, 16, 14):
                    nc.sync.dma_start(
                        x_sb[:, r0 : r0 + nr, :], x_d[i, :, r0 : r0 + nr, :]
                    )
                    nc.scalar.activation(
                        xs3[:, 1 + r0 : 1 + r0 + nr, 1 : W + 1],
                        x_sb[:, r0 : r0 + nr, :],
                        SIGN,
                    )
                    shift_copy(xs, 1 + r0, nr)
                    r0 += nr

                ts = tpool.tile([C, 2 * HP * RW], FP8)
                ts3 = ts[:, 0:SHIFT].rearrange("p (h w) -> p h w", w=RW)
                zero_pads(ts, ts3)

                for k in range(N_CHUNKS):
                    h0 = k * CHUNK_ROWS
                    ps1 = pspool.tile([C, NFLAT], F32, tag="ps1")
                    conv_chunk(ps1, xs, 0, h0)
                    # bn1 + sign (hardtanh folded into sign) -> conv2 input
                    ps1v = ps1.rearrange("p (h w) -> p h w", w=RW)[:, :, 0:W]
                    nc.scalar.activation(
                        ts3[:, 1 + h0 : 1 + h0 + CHUNK_ROWS, 1 : W + 1],
                        ps1v,
                        SIGN,
                        bias=bn_sb[:, 1:2],
                        scale=bn_sb[:, 0:1],
                    )
                    shift_copy(ts, 1 + h0, CHUNK_ROWS)

                for k in range(N_CHUNKS):
                    h0 = k * CHUNK_ROWS
                    # residual + bn2 bias, precomputed on ACT (no psum dep, so
                    # it runs early and keeps the DVE eviction chain short)
                    xb2 = opool.tile([C, CHUNK_ROWS, W], F32, tag="xb2")
                    nc.scalar.activation(
                        xb2[:], x_sb[:, h0 : h0 + CHUNK_ROWS, :],
                        mybir.ActivationFunctionType.Identity,
                        bias=bn_sb[:, 3:4],
                    )
                    ps2 = pspool.tile([C, NFLAT], F32, tag="ps2")
                    conv_chunk(ps2, ts, 1, h0)
                    ps2v = ps2.rearrange("p (h w) -> p h w", w=RW)[:, :, 0:W]
                    o = opool.tile([C, CHUNK_ROWS, W], F32)
                    # out = clip(ps2*inv2 + (x + b2), -1, 1) in two DVE ops
                    nc.vector.scalar_tensor_tensor(
                        o[:], ps2v, bn_sb[:, 2:3], xb2[:],
                        op0=mybir.AluOpType.mult, op1=mybir.AluOpType.add,
                    )
                    nc.vector.tensor_scalar(
                        o[:], o[:], 1.0, -1.0,
                        op0=mybir.AluOpType.min, op1=mybir.AluOpType.max,
                    )
                    nc.sync.dma_start(y_d[i, :, h0 : h0 + CHUNK_ROWS, :], o[:])

    nc.compile()
    return nc


def _get_nc():
    global _NC_CACHE
    if _NC_CACHE is None:
        _NC_CACHE = _build_nc()
    return _NC_CACHE


def kernel(
    x, w1, w2, gamma1, beta1, mean1, var1, gamma2, beta2, mean2, var2,
    trace=False,
):
    x = np.ascontiguousarray(np.asarray(x, dtype=np.float32))
    w1 = np.asarray(w1, dtype=np.float32)
    w2 = np.asarray(w2, dtype=np.float32)

    # fold BN exactly as the reference does (f32 throughout)
    def fold(gamma, beta, mean, var):
        inv = (np.asarray(gamma, np.float32)
               / np.sqrt(np.asarray(var, np.float32) + np.float32(EPS)))
        b = np.asarray(beta, np.float32) - np.asarray(mean, np.float32) * inv
        return inv.astype(np.float32), b.astype(np.float32)

    inv1, b1 = fold(gamma1, beta1, mean1, var1)
    inv2, b2 = fold(gamma2, beta2, mean2, var2)
    bn_np = np.stack([inv1, b1, inv2, b2], axis=1).astype(np.float32)  # [C,4]

    # fp8 weight tables; per conv: 4 DoubleRow pair tables then 1 normal.
    # DR c=0..2: w_np[k, co + c*256 + j*128 + m] = sign(w[m,k,j,c]), j=row 0/1
    # DR #3:     pairs (r2,c0) j=0 and (r2,c1) j=1 at co+768
    # normal:    (r2,c2) at co+1024
    w_np = np.empty((C, WCOLS), dtype=ml_dtypes.float8_e4m3fn)
    for conv_idx, w in enumerate((w1, w2)):
        ws = np.sign(w).astype(ml_dtypes.float8_e4m3fn)  # [O, Cin, 3, 3]
        co = conv_idx * 1152
        for c in range(3):
            for j in range(2):
                w_np[:, co + c * 256 + j * 128 : co + c * 256 + (j + 1) * 128] = (
                    ws[:, :, j, c].T
                )
        w_np[:, co + 768 : co + 896] = ws[:, :, 2, 0].T
        w_np[:, co + 896 : co + 1024] = ws[:, :, 2, 1].T
        w_np[:, co + 1024 : co + 1152] = ws[:, :, 2, 2].T

    nc = _get_nc()
    in_maps = [
        {"x": x[i * BL : (i + 1) * BL], "w": w_np, "bn": bn_np}
        for i in range(N_CORES)
    ]
    res = run_bass_kernel_spmd(
        nc, in_maps, core_ids=list(range(N_CORES)), trace=trace
    )
    y = np.concatenate([res.results[i]["y"] for i in range(N_CORES)], axis=0)
    if trace:
        return y.astype(np.float32), res
    return y.astype(np.float32)


# revision 16
# speedup vs baseline: 1.7496x; 1.7496x over previous
"""Trainium2 Bass kernel for nn_BasicBlock (binarized ResNet basic block).

Computation (see problem reference):
    residual = x
    out = psum_conv3x3(sign(x), sign(w1))        # 3x3 'same' conv, saturating acc
    out = bn1(out); out = hardtanh(out)
    out = psum_conv3x3(sign(out), sign(w2))
    out = bn2(out); out = out + residual; out = hardtanh(out)

Key facts exploited:
  * C=128 channels = one GROUP, 9 taps of |partial| <= 128 each, so the
    running accumulator magnitude is <= 9*128 = 1152 < THRESH=8000: the
    saturation clip NEVER binds. The conv is a plain 3x3 conv over sign
    values, all arithmetic exact small integers -> freely reorderable and
    exactly representable in fp8e4/bf16 inputs with fp32 PSUM accumulation.
  * sign(hardtanh(v)) == sign(v), so the first hardtanh can be folded into
    the sign feeding conv2.
  * Each conv = 9 shifted-window taps (K=C=128 on partitions) into one PSUM
    accumulation group over a zero-padded row-stride-64 fp8 sign image:
    the 6 taps of rows 0..1 run as 3 fp8 DoubleRow matmuls (two vertically
    adjacent taps share one matmul: planes at +64 elements), the 3 taps of
    row 2 as normal fp8 matmuls.

Sharding: data-parallel over batch: 64 images -> 8 cores x 8 images.
"""

import sys

sys.path.insert(0, "/opt/trn_rl_repo")

import numpy as np
import ml_dtypes

import concourse.bass as bass
import concourse.bacc as bacc
import concourse.mybir as mybir
import concourse.tile as tile
from concourse.bass_utils import run_bass_kernel_spmd

# ---------------------------------------------------------------- constants

N_CORES = 8
B, C, H, W = 64, 128, 56, 56
BL = B // N_CORES            # images per core
HP = H + 2                   # padded rows
RW = 64                      # padded row width (stride): 56 valid + pads,
                             # 64 so the DoubleRow plane step (+RW) is 16-aligned
CHUNK_ROWS = 8               # output rows per PSUM chunk
NFLAT = CHUNK_ROWS * RW      # 512 flat psum columns per chunk (one bank)
N_CHUNKS = H // CHUNK_ROWS   # 7
EPS = 1e-5
SHIFT = (H + 2) * RW         # offset of the col-shifted copy inside xs/ts
WCOLS = 2 * (4 * 256 + 128)  # fp8 weight table columns (2 convs x 1152)

F32 = mybir.dt.float32
FP8 = mybir.dt.float8e4

_NC_CACHE = None


def _build_nc():
    """Build the per-core Bass module (same NEFF on all 8 cores)."""
    nc = bacc.Bacc("TRN2", debug=False)

    x_d = nc.dram_tensor("x", [BL, C, H, W], F32, kind="ExternalInput").ap()
    # host-prepped fp8 weight tables, per conv: 3 DoubleRow pair tables
    # [cin, 2*cout] for (r0,r1) at c=0,1,2 then 3 normal tables [cin, cout]
    # for r=2 at c=0,1,2
    w_d = nc.dram_tensor("w", [C, WCOLS], FP8, kind="ExternalInput").ap()
    # folded BN params per channel: [:,0]=inv1 [:,1]=b1 [:,2]=inv2 [:,3]=b2
    bn_d = nc.dram_tensor("bn", [C, 4], F32, kind="ExternalInput").ap()
    y_d = nc.dram_tensor("y", [BL, C, H, W], F32, kind="ExternalOutput").ap()

    SIGN = mybir.ActivationFunctionType.Sign
    DR = mybir.MatmulPerfMode.DoubleRow

    with tile.TileContext(nc) as tc:
        with (
            tc.tile_pool(name="const", bufs=1) as cpool,
            tc.tile_pool(name="xf32", bufs=2) as xpool,
            tc.tile_pool(name="xsign", bufs=2) as spool,
            tc.tile_pool(name="tmid", bufs=2) as tpool,
            tc.tile_pool(name="outs", bufs=3) as opool,
            tc.tile_pool(name="psum", bufs=3, space="PSUM") as pspool,
        ):
            w_sb = cpool.tile([C, WCOLS], FP8)
            nc.sync.dma_start(w_sb[:], w_d[:])
            bn_sb = cpool.tile([C, 4], F32)
            nc.sync.dma_start(bn_sb[:], bn_d[:])

            def conv_chunk(ps, src, conv_idx, h0):
                """One output chunk: 4 DoubleRow + 1 normal fp8 matmuls.

                DR c=0..2 pairs the vertically adjacent taps (r0,c)+(r1,c)
                (planes at +RW). DR #3 pairs (r2,c0)+(r2,c1) using the
                col-shifted copy at +SHIFT. Tap (r2,c2) is a normal matmul.
                """
                co = conv_idx * 1152
                ps3 = ps.rearrange("p (h w) -> p h w", w=RW)
                pout = ps3[:, :, 0:W]
                for c in range(3):
                    rhs = bass.AP(
                        tensor=src.tensor,
                        offset=src.offset + h0 * RW + c,
                        ap=[src.ap[0], [RW, 2], [RW, CHUNK_ROWS], [1, W]],
                    )
                    lhsT = w_sb[:, co + c * 256 : co + (c + 1) * 256].rearrange(
                        "p (j m) -> p j m", j=2
                    )
                    nc.tensor.matmul(
                        pout, lhsT, rhs, start=(c == 0), stop=False,
                        perf_mode=DR, skip_group_check=True,
                    )
                rhs = bass.AP(
                    tensor=src.tensor,
                    offset=src.offset + (h0 + 2) * RW,
                    ap=[src.ap[0], [SHIFT, 2], [RW, CHUNK_ROWS], [1, W]],
                )
                lhsT = w_sb[:, co + 768 : co + 1024].rearrange(
                    "p (j m) -> p j m", j=2
                )
                nc.tensor.matmul(
                    pout, lhsT, rhs, start=False, stop=False,
                    perf_mode=DR, skip_group_check=True,
                )
                rhs = bass.AP(
                    tensor=src.tensor,
                    offset=src.offset + (h0 + 2) * RW + 2,
                    ap=[src.ap[0], [RW, CHUNK_ROWS], [1, W]],
                )
                nc.tensor.matmul(
                    pout, w_sb[:, co + 1024 : co + 1152],
                    rhs, start=False, stop=True, skip_group_check=True,
                )

            def shift_copy(buf, row0, nrows):
                """Build sh[h, w] = main[h, w+1] for rows [row0, row0+nrows)."""
                src = bass.AP(
                    tensor=buf.tensor,
                    offset=buf.offset + row0 * RW + 1,
                    ap=[buf.ap[0], [1, nrows * RW]],
                )
                dst = bass.AP(
                    tensor=buf.tensor,
                    offset=buf.offset + SHIFT + row0 * RW,
                    ap=[buf.ap[0], [1, nrows * RW]],
                )
                nc.vector.tensor_copy(dst, src)

            def zero_pads(buf, buf3):
                nc.gpsimd.memset(buf3[:, 0, :], 0.0)
                nc.gpsimd.memset(buf3[:, HP - 1, :], 0.0)
                nc.gpsimd.memset(buf3[:, 1 : HP - 1, 0:1], 0.0)
                nc.gpsimd.memset(buf3[:, 1 : HP - 1, W + 1 : RW], 0.0)
                # last padded row of the shifted copy is never built by
                # shift_copy; it is all pad-derived, so zero it once
                nc.gpsimd.memset(buf[:, SHIFT + (HP - 1) * RW : 2 * SHIFT], 0.0)

            for i in range(BL):
                # DMA + sign in 14-row pieces so conv1 can start after the
                # first piece instead of after the whole 1.6 MB transfer.
                x_sb = xpool.tile([C, H, W], F32)
                xs = spool.tile([C, 2 * HP * RW], FP8)
                xs3 = xs[:, 0:SHIFT].rearrange("p (h w) -> p h w", w=RW)
                zero_pads(xs, xs3)
                r0 = 0
                for nr in (10, 16, 16, 14):
                    nc.sync.dma_start(
                        x_sb[:, r0 : r0 + nr, :], x_d[i, :, r0 : r0 + nr, :]
                    )
                    nc.scalar.activation(
                        xs3[:, 1 + r0 : 1 + r0 + nr, 1 : W + 1],
                        x_sb[:, r0 : r0 + nr, :],
                        SIGN,
                    )
                    shift_copy(xs, 1 + r0, nr)
                    r0 += nr

                ts = tpool.tile([C, 2 * HP * RW], FP8)
                ts3 = ts[:, 0:SHIFT].rearrange("p (h w) -> p h w", w=RW)
                zero_pads(ts, ts3)

                for k in range(N_CHUNKS):
                    h0 = k * CHUNK_ROWS
                    ps1 = pspool.tile([C, NFLAT], F32, tag="ps1")
                    conv_chunk(ps1, xs, 0, h0)
                    # bn1 + sign (hardtanh folded into sign) -> conv2 input
                    ps1v = ps1.rearrange("p (h w) -> p h w", w=RW)[:, :, 0:W]
                    nc.scalar.activation(
                        ts3[:, 1 + h0 : 1 + h0 + CHUNK_ROWS, 1 : W + 1],
                        ps1v,
                        SIGN,
                        bias=bn_sb[:, 1:2],
                        scale=bn_sb[:, 0:1],
                    )
                    shift_copy(ts, 1 + h0, CHUNK_ROWS)

                for k in range(N_CHUNKS):
                    h0 = k * CHUNK_ROWS
                    # residual + bn2 bias, precomputed on ACT (no psum dep, so
                    # it runs early and keeps the DVE eviction chain short)
                    xb2 = opool.tile([C, CHUNK_ROWS, W], F32, tag="xb2")
                    nc.scalar.activation(
                        xb2[:], x_sb[:, h0 : h0 + CHUNK_ROWS, :],
                        mybir.ActivationFunctionType.Identity,
                        bias=bn_sb[:, 3:4],
                    )
                    ps2 = pspool.tile([C, NFLAT], F32, tag="ps2")
                    conv_chunk(ps2, ts, 1, h0)
                    ps2v = ps2.rearrange("p (h w) -> p h w", w=RW)[:, :, 0:W]
                    o = opool.tile([C, CHUNK_ROWS, W], F32)
                    # out = clip(ps2*inv2 + (x + b2), -1, 1) in two DVE ops
                    nc.vector.scalar_tensor_tensor(
                        o[:], ps2v, bn_sb[:, 2:3], xb2[:],
                        op0=mybir.AluOpType.mult, op1=mybir.AluOpType.add,
                    )
                    nc.vector.tensor_scalar(
                        o[:], o[:], 1.0, -1.0,
                        op0=mybir.AluOpType.min, op1=mybir.AluOpType.max,
                    )
                    nc.sync.dma_start(y_d[i, :, h0 : h0 + CHUNK_ROWS, :], o[:])

    nc.compile()
    return nc


def _get_nc():
    global _NC_CACHE
    if _NC_CACHE is None:
        _NC_CACHE = _build_nc()
    return _NC_CACHE


def kernel(
    x, w1, w2, gamma1, beta1, mean1, var1, gamma2, beta2, mean2, var2,
    trace=False,
):
    x = np.ascontiguousarray(np.asarray(x, dtype=np.float32))
    w1 = np.asarray(w1, dtype=np.float32)
    w2 = np.asarray(w2, dtype=np.float32)

    # fold BN exactly as the reference does (f32 throughout)
    def fold(gamma, beta, mean, var):
        inv = (np.asarray(gamma, np.float32)
               / np.sqrt(np.asarray(var, np.float32) + np.float32(EPS)))
        b = np.asarray(beta, np.float32) - np.asarray(mean, np.float32) * inv
        return inv.astype(np.float32), b.astype(np.float32)

    inv1, b1 = fold(gamma1, beta1, mean1, var1)
    inv2, b2 = fold(gamma2, beta2, mean2, var2)
    bn_np = np.stack([inv1, b1, inv2, b2], axis=1).astype(np.float32)  # [C,4]

    # fp8 weight tables; per conv: 4 DoubleRow pair tables then 1 normal.
    # DR c=0..2: w_np[k, co + c*256 + j*128 + m] = sign(w[m,k,j,c]), j=row 0/1
    # DR #3:     pairs (r2,c0) j=0 and (r2,c1) j=1 at co+768
    # normal:    (r2,c2) at co+1024
    w_np = np.empty((C, WCOLS), dtype=ml_dtypes.float8_e4m3fn)
    for conv_idx, w in enumerate((w1, w2)):
        ws = np.sign(w).astype(ml_dtypes.float8_e4m3fn)  # [O, Cin, 3, 3]
        co = conv_idx * 1152
        for c in range(3):
            for j in range(2):
                w_np[:, co + c * 256 + j * 128 : co + c * 256 + (j + 1) * 128] = (
                    ws[:, :, j, c].T
                )
        w_np[:, co + 768 : co + 896] = ws[:, :, 2, 0].T
        w_np[:, co + 896 : co + 1024] = ws[:, :, 2, 1].T
        w_np[:, co + 1024 : co + 1152] = ws[:, :, 2, 2].T

    nc = _get_nc()
    in_maps = [
        {"x": x[i * BL : (i + 1) * BL], "w": w_np, "bn": bn_np}
        for i in range(N_CORES)
    ]
    res = run_bass_kernel_spmd(
        nc, in_maps, core_ids=list(range(N_CORES)), trace=trace
    )
    y = np.concatenate([res.results[i]["y"] for i in range(N_CORES)], axis=0)
    if trace:
        return y.astype(np.float32), res
    return y.astype(np.float32)
